# revision 1
# baseline (speedup 1.0000x reference)
"""Trainium2 Bass kernel for the lipsnet CustomModel problem.

Math: the reference computes, per sample,
    jac_norm = ||D3 W3 D2 W2 D1 W1||_F      (Di = diag(relu'(pi)))
    out = tanh(k_out * f_out / (jac_norm + 1e-4))
Key identity used here:  with G = W1 W1^T = L L^T (host eigen factorization),
    ||D3 W3 D2 W2 D1 W1||_F^2 = ||D3 W3 D2 W2 D1 L||_F^2
                              = sum_c || D3 W3 D2 (M_c @ d1) ||^2
where M_c[j,l] = W2[j,l] * L[l,c] are 85 host-precomputed stationary
matrices and d1/d2/d3 are the per-sample binary relu masks.  Every
per-sample 85x85x85 contraction becomes a stationary-weight matmul with
the mask tensor [85, S] as the moving operand.

PE-cycle reductions over the first working version:
  * forward layers run in float32r (1 col/cycle at >=256 moving cols vs
    4 for plain fp32; inputs produced by fp32r-rounding DVE/ACT writes)
  * stage-1 (M_c @ d1) runs in fp8e4 DoubleRow perf mode: the 85-long
    contraction is split into two 43-row k-tiles stacked in the free dim,
    halving PE columns (mall prescaled x16 to stay in e4m3 normal range)
  * the square-accumulate over c runs as fp8 DoubleRow identity matmuls
    that fold PAIRS of sq_c tiles per pass (sq = (4*pr)^2 in e4m3),
    quartering the accumulation matmul cycles; the 64^2 scale is divided
    back out in the finale.

Sharding: pure data parallel over the batch dim, 8 NeuronCores, weights
replicated.  kernel() takes FULL inputs and returns the FULL output.
"""

import os
from contextlib import ExitStack

import numpy as np

import concourse.bass as bass
import concourse.bacc as bacc
import concourse.mybir as mybir
import concourse.tile as tile

F32 = mybir.dt.float32
F32R = mybir.dt.float32r
BF16 = mybir.dt.bfloat16
F8 = mybir.dt.float8e4
AF = mybir.ActivationFunctionType
OP = mybir.AluOpType
DR = mybir.MatmulPerfMode.DoubleRow

B = 8192
OBS = 64
ACTD = 16
H = 128
COMP = 85
KH = 43            # ceil(85/2): DoubleRow k-tile height
KS = 32
MP = 96            # DoubleRow stationary free-dim pad (16B-aligned stride)
NCORES = 8
S = B // NCORES        # 1024 samples per core
NB = S // 128          # 8 sample blocks of 128
CH = 512               # matmul moving-operand chunk (one PSUM bank of f32)
EPS = 1e-4
MSC = 16.0             # host prescale on mall (keeps e4m3 in normal range)
QSC = 4.0              # ACT square input prescale: sq = (QSC*MSC*pr_true)^2
JSC = MSC * QSC        # jn_scaled = JSC * jn_true

# name -> (shape, pack idx) of every replicated weight, packed host-side
# into three [128, N] arrays (f32 / bf16 / fp8e4) so the kernel needs 3 DMAs
_WSLOTS = {
    "ow1T": ([OBS, H], 0), "ob1": ([H, 1], 0), "ow2T": ([H, H], 0),
    "ob2": ([H, 1], 0), "aw1T": ([ACTD, H], 0), "ab1": ([H, 1], 0),
    "aw2T": ([H, H], 0), "ab2": ([H, 1], 0),
    "kw1Ta": ([H, KS], 0), "kw1Tb": ([H, KS], 0), "kb1": ([KS, 1], 0),
    "kw2T": ([KS, KS // 2], 0), "kb2": ([KS // 2, 1], 0),
    "kw3T": ([KS // 2, 1], 0), "kb3": ([1, 1], 0),
    "mw1Ta": ([H, COMP], 0), "mw1Tb": ([H, COMP], 0), "mb1": ([COMP, 1], 0),
    "mw2T": ([COMP, COMP], 0), "mb2": ([COMP, 1], 0),
    "mw3T": ([COMP, COMP], 0), "mb3": ([COMP, 1], 0),
    "ones": ([COMP, 1], 0), "iden": ([H, H], 0),
    "mw3Tb": ([COMP, COMP], 1), "idenb": ([COMP, COMP], 1),
    "onesb": ([COMP, 1], 1),
    "mallDR": ([KH, COMP * 2 * MP], 2), "idenDR": ([COMP, 2 * MP], 2),
}
_OFFS = {}
_NCOLS = [0, 0, 0]
for _n, (_shp, _b) in _WSLOTS.items():
    _OFFS[_n] = _NCOLS[_b]
    _NCOLS[_b] += _shp[1]

def host_prep(inputs):
    """Host-side weight preprocessing + packing (pure numpy, all tiny)."""
    import ml_dtypes
    f = lambda a: np.ascontiguousarray(np.asarray(a, dtype=np.float32))
    W1, W2, W3 = f(inputs["mw1"]), f(inputs["mw2"]), f(inputs["mw3"])
    G = (W1 @ W1.T).astype(np.float64)
    lam, U = np.linalg.eigh(G)
    L = (U * np.sqrt(np.clip(lam, 0.0, None))).astype(np.float32)  # G = L L^T
    # M[l, c, m] = W2[m, l] * L[l, c]   (stage-1 stationary, c-th slab)
    M = W2.T[:, None, :] * L[:, :, None]          # [85, 85, 85]
    # DoubleRow split-k layout: A[k, c, t, m] = MSC * M[k + 43t, c, m]
    A = np.zeros((KH, COMP, 2, MP), np.float32)
    A[:, :, 0, 0:COMP] = M[0:KH]
    A[0:COMP - KH, :, 1, 0:COMP] = M[KH:COMP]
    mallDR = (MSC * A).reshape(KH, COMP * 2 * MP)
    # DoubleRow identity for the paired square-accumulate
    I2 = np.zeros((COMP, 2, MP), np.float32)
    for t in range(2):
        I2[np.arange(COMP), t, np.arange(COMP)] = 1.0
    idenDR = I2.reshape(COMP, 2 * MP)
    vals = {
        "ow1T": f(inputs["ow1"]).T, "ob1": f(inputs["ob1"]).reshape(H, 1),
        "ow2T": f(inputs["ow2"]).T, "ob2": f(inputs["ob2"]).reshape(H, 1),
        "aw1T": f(inputs["aw1"]).T, "ab1": f(inputs["ab1"]).reshape(H, 1),
        "aw2T": f(inputs["aw2"]).T, "ab2": f(inputs["ab2"]).reshape(H, 1),
        "kw1Ta": f(inputs["kw1"]).T[:H], "kw1Tb": f(inputs["kw1"]).T[H:],
        "kb1": f(inputs["kb1"]).reshape(KS, 1),
        "kw2T": f(inputs["kw2"]).T, "kb2": f(inputs["kb2"]).reshape(KS // 2, 1),
        "kw3T": f(inputs["kw3"]).T, "kb3": f(inputs["kb3"]).reshape(1, 1),
        "mw1Ta": W1.T[:H], "mw1Tb": W1.T[H:],
        "mb1": f(inputs["mb1"]).reshape(COMP, 1),
        "mw2T": W2.T, "mb2": f(inputs["mb2"]).reshape(COMP, 1),
        "mw3T": W3.T, "mb3": f(inputs["mb3"]).reshape(COMP, 1),
        "ones": np.ones((COMP, 1), np.float32),
        "iden": np.eye(H, dtype=np.float32),
        "mw3Tb": W3.T,
        "idenb": np.eye(COMP, dtype=np.float32),
        "onesb": np.ones((COMP, 1), np.float32),
        "mallDR": mallDR, "idenDR": idenDR,
    }
    packs = [np.zeros((128, _NCOLS[0]), np.float32),
             np.zeros((128, _NCOLS[1]), ml_dtypes.bfloat16),
             np.zeros((128, _NCOLS[2]), ml_dtypes.float8_e4m3)]
    for n, (shp, b) in _WSLOTS.items():
        o = _OFFS[n]
        packs[b][:shp[0], o:o + shp[1]] = vals[n]
    return {"wpack32": packs[0], "wpack16": packs[1], "wpack8": packs[2]}


def build_nc(reps=1):
    nc = bacc.Bacc()

    obs_d = nc.declare_dram_parameter("obs", [S, OBS], F32, isOutput=False)
    act_d = nc.declare_dram_parameter("action", [S, ACTD], F32, isOutput=False)
    wp32_d = nc.declare_dram_parameter("wpack32", [128, _NCOLS[0]], F32,
                                       isOutput=False)
    wp16_d = nc.declare_dram_parameter("wpack16", [128, _NCOLS[1]], BF16,
                                       isOutput=False)
    wp8_d = nc.declare_dram_parameter("wpack8", [128, _NCOLS[2]], F8,
                                      isOutput=False)
    tick_d = nc.declare_dram_parameter("tick", [1, 1], F32, isOutput=False)
    out_d = nc.declare_dram_parameter("out", [S, COMP], F32, isOutput=True)

    with tile.TileContext(nc) as tc, ExitStack() as ctx:
        wp = ctx.enter_context(tc.tile_pool(name="weights", bufs=1))
        ap = ctx.enter_context(tc.tile_pool(name="acts", bufs=1))
        zp = ctx.enter_context(tc.tile_pool(name="zbuf", bufs=4))
        sqp = ctx.enter_context(tc.tile_pool(name="sqbuf", bufs=3))
        outp = ctx.enter_context(tc.tile_pool(name="outbuf", bufs=3))
        smp = ctx.enter_context(tc.tile_pool(name="small", bufs=16))
        psA = ctx.enter_context(tc.tile_pool(name="psA", bufs=3, space="PSUM"))
        psC = ctx.enter_context(tc.tile_pool(name="psC", bufs=1, space="PSUM"))

        # ---- load weights (3 packed DMAs), expose per-weight slice views ----
        wp32 = wp.tile([128, _NCOLS[0]], F32, tag="wp32", name="wp32")
        wp16 = wp.tile([128, _NCOLS[1]], BF16, tag="wp16", name="wp16")
        wp8 = wp.tile([128, _NCOLS[2]], F8, tag="wp8", name="wp8")
        nc.sync.dma_start(wp32[:], wp32_d[:])
        w = {}
        for name, (shp, b) in _WSLOTS.items():
            o = _OFFS[name]
            w[name] = [wp32, wp16, wp8][b][0:shp[0], o:o + shp[1]]
        # stage-1 DoubleRow stationary slabs, viewed [43, c, 2, 85]
        mallDR = wp8[0:KH, _OFFS["mallDR"]:_OFFS["mallDR"] + COMP * 2 * MP]
        mallDR = mallDR.rearrange("k (c t m) -> k c t m", c=COMP, t=2, m=MP)
        idenDR = w["idenDR"].rearrange("k (t m) -> k t m", t=2, m=MP)

        tick_sb = wp.tile([1, 1], F32, tag="tick_sb", name="tick_sb")
        nc.sync.dma_start(tick_sb[:], tick_d[:])

        # forward-layer stationaries, rounded on-chip to float32r (the BIR
        # verifier requires fp32r matmul inputs to come from a rounding
        # instruction; a DVE copy with fp32r output qualifies)
        _RW = ["ow1T", "ow2T", "aw1T", "aw2T", "kw1Ta", "kw1Tb", "kw2T",
               "kw3T", "mw1Ta", "mw1Tb", "mw2T", "mw3T"]
        _rcols = sum(_WSLOTS[n][0][1] for n in _RW)
        wpR = wp.tile([128, _rcols], F32R, tag="wpR", name="wpR")
        wR = {}
        _off = 0
        for n in _RW:
            shp = _WSLOTS[n][0]
            wR[n] = wpR[0:shp[0], _off:_off + shp[1]]
            nc.vector.tensor_copy(wR[n], w[n])
            _off += shp[1]

        # ---- load + transpose obs/action into [feat, S] layout ----
        for _rep in range(reps):
            obs_sb = ap.tile([128, NB, OBS], F32, tag="obs_sb")
            act_sb = ap.tile([128, NB, ACTD], F32, tag="act_sb")
            nc.sync.dma_start(obs_sb[:],
                              obs_d[:].rearrange("(nb p) f -> p nb f", p=128))
            nc.sync.dma_start(act_sb[:],
                              act_d[:].rearrange("(nb p) f -> p nb f", p=128))
            # collapse the many DMA-queue semaphores into one barrier so no
            # matmul needs more than one sync wait (walrus S3_LW limit)
            tc.strict_bb_all_engine_barrier()
            # the fp8/bf16 packs are only needed at J-loop start; issued
            # after the barrier so the forward overlaps them
            nc.sync.dma_start(wp16[:], wp16_d[:])
            nc.sync.dma_start(wp8[:], wp8_d[:])

            obst = ap.tile([OBS, S], F32R, tag="obst")
            actt = ap.tile([ACTD, S], F32R, tag="actt")
            for nb in range(NB):
                pt = psA.tile([OBS, 128], F32, tag="a")
                nc.tensor.transpose(pt[:], obs_sb[:, nb, :], w["iden"][:])
                nc.vector.tensor_copy(obst[:, nb * 128:(nb + 1) * 128], pt[:])
                pt2 = psA.tile([ACTD, 128], F32, tag="a")
                nc.tensor.transpose(pt2[:], act_sb[:, nb, :], w["iden"][:])
                nc.vector.tensor_copy(actt[:, nb * 128:(nb + 1) * 128], pt2[:])

            # ---- forward layers ([feat, S], f32r chunked matmuls + ACT) ----
            def layer(dst, dst_sl, terms, bias, func, p):
                # dst[dst_sl] = func(sum_i lhsT_i.T @ rhs_i + bias), chunked over S
                for ch in range(S // CH):
                    sl = slice(ch * CH, (ch + 1) * CH)
                    pt = p.tile([terms[0][0].shape[-1], CH], F32, tag="a", name="pt")
                    n = len(terms)
                    for i, (lhsT, rhs) in enumerate(terms):
                        nc.tensor.matmul(pt[:], lhsT[:], rhs[:, sl],
                                         start=(i == 0), stop=(i == n - 1))
                    dsl = dst[:, sl] if dst_sl is None else dst[dst_sl, sl]
                    if func == AF.Relu:
                        nc.vector.tensor_scalar(out=dsl, in0=pt[:], scalar1=bias[:],
                                                scalar2=0.0, op0=OP.add, op1=OP.max)
                    else:
                        nc.scalar.activation(dsl, pt[:], func, bias=bias[:])

            oh1 = ap.tile([H, S], F32R, tag="oh1")
            layer(oh1, None, [(wR["ow1T"], obst)], w["ob1"], AF.Relu, psA)
            of = ap.tile([H, S], F32R, tag="of")
            layer(of, None, [(wR["ow2T"], oh1)], w["ob2"], AF.Relu, psA)
            ah1 = ap.tile([H, S], F32R, tag="ah1")
            layer(ah1, None, [(wR["aw1T"], actt)], w["ab1"], AF.Relu, psA)
            af = ap.tile([H, S], F32R, tag="af")
            layer(af, None, [(wR["aw2T"], ah1)], w["ab2"], AF.Relu, psA)

            k1 = ap.tile([KS, S], F32R, tag="k1")
            layer(k1, None, [(wR["kw1Ta"], of), (wR["kw1Tb"], af)], w["kb1"], AF.Tanh, psA)
            k2 = ap.tile([KS // 2, S], F32R, tag="k2")
            layer(k2, None, [(wR["kw2T"], k1)], w["kb2"], AF.Tanh, psA)

            # k_out = softplus(kw3 @ k2 + kb3) = ln(1 + exp(.)) via Exp then Ln(x+1)
            kexp = ap.tile([1, S], F32, tag="kexp")
            layer(kexp, None, [(wR["kw3T"], k2)], w["kb3"], AF.Exp, psA)
            kout = ap.tile([1, S], F32, tag="kout")
            nc.scalar.activation(kout[:], kexp[:], AF.Ln, bias=1.0)

            h1 = ap.tile([COMP, S], F32R, tag="h1")
            layer(h1, None, [(wR["mw1Ta"], of), (wR["mw1Tb"], af)], w["mb1"], AF.Relu, psA)
            # d1 in DoubleRow split-k layout [43, 2, S]: (k, t, s) = d1[k+43t, s]
            d1dr = ap.tile([KH, 2, S], F8, tag="d1dr")
            d1lo = ap.tile([COMP, S], F8, tag="d1lo")
            nc.vector.tensor_scalar(out=d1lo[:], in0=h1[:], scalar1=0.0, scalar2=None,
                                    op0=OP.is_gt)
            nc.vector.tensor_copy(d1dr[:, 0, :], d1lo[0:KH, :])
            # zero the whole t=1 slice (engine ops need partition base 0/32/64/96),
            # then the partition-shift DMA overwrites rows 0..41 with d1[43:85]
            nc.vector.memset(d1dr[:, 1, :], 0.0)
            nc.sync.dma_start(d1dr[0:COMP - KH, 1, :], d1lo[KH:COMP, :])
            h2 = ap.tile([COMP, S], F32R, tag="h2")
            layer(h2, None, [(wR["mw2T"], h1)], w["mb2"], AF.Relu, psA)
            d2 = ap.tile([COMP, S], F32, tag="d2")
            nc.vector.tensor_scalar(out=d2[:], in0=h2[:], scalar1=0.0,
                                    scalar2=None, op0=OP.is_gt)
            fout = ap.tile([COMP, S], F32, tag="fout")
            layer(fout, None, [(wR["mw3T"], h2)], w["mb3"], AF.Relu, psA)
            d3 = ap.tile([COMP, S], F32, tag="d3")
            nc.vector.tensor_scalar(out=d3[:], in0=fout[:], scalar1=0.0,
                                    scalar2=None, op0=OP.is_gt)

            # ---- Jacobian-norm loop over the 85 columns of L ----
            accp = psC.tile([MP, S], F32, tag="c")   # persistent PSUM accumulator
            ACCs = ap.tile([COMP, S], F32, tag="ACCs")  # SBUF spill of acc groups
            NPAIR = COMP // 2                          # 42 fp8-paired acc matmuls
            GRP = 21                                   # acc-group length (pairs)
            spills = [0]

            def spill():
                if spills[0] == 0:
                    nc.vector.tensor_copy(ACCs[:], accp[0:COMP, :])
                else:
                    nc.vector.tensor_tensor(ACCs[:], accp[0:COMP, :], ACCs[:], OP.add)
                spills[0] += 1

            def acc_pair(sqt, n):
                # accp += sq[t=0] + sq[t=1] (fp8 DoubleRow identity matmul)
                for ch in range(S // CH):
                    sl = slice(ch * CH, (ch + 1) * CH)
                    nc.tensor.matmul(accp[:, sl], idenDR[:], sqt[:, :, sl],
                                     start=(n % GRP == 0),
                                     stop=(n % GRP == GRP - 1 or n == NPAIR - 1),
                                     perf_mode=DR, skip_group_check=True)
                if n % GRP == GRP - 1 or n == NPAIR - 1:
                    spill()

            tc.strict_bb_all_engine_barrier()
            # software pipeline: py prefetched one c ahead of the DVE mask,
            # squares accumulated one pair behind, so PE never heads-of-line
            # blocks the mask -> pr -> py -> mask cycle
            pys = {}

            def emit_py(c):
                # py_c = (MSC*M_c) @ d1, via fp8 DoubleRow split-k matmul
                t = psA.tile([MP, S], F32, tag="a", name="py")
                for ch in range(S // CH):
                    sl = slice(ch * CH, (ch + 1) * CH)
                    nc.tensor.matmul(t[:, sl], mallDR[:, c, :, :],
                                     d1dr[:, :, sl], start=True, stop=True,
                                     perf_mode=DR)
                pys[c] = t

            emit_py(0)
            pend = []
            sq_cur = [None]
            for c in range(COMP):
                z = zp.tile([COMP, S], BF16, tag="z")
                nc.vector.tensor_tensor(z[:], pys.pop(c)[0:COMP, :], d2[:], OP.mult)
                if c + 1 < COMP:
                    emit_py(c + 1)
                if len(pend) == 2:
                    acc_pair(*pend.pop(0))
                pr = psA.tile([COMP, S], F32, tag="a", name="pr")
                for ch in range(S // CH):
                    sl = slice(ch * CH, (ch + 1) * CH)
                    nc.tensor.matmul(pr[:, sl], w["mw3Tb"][:], z[:, sl],
                                     start=True, stop=True)
                if c % 2 == 0:
                    sq_cur[0] = sqp.tile([COMP, 2, S], F8, tag="sq", name="sqpair")
                if c == COMP - 1:
                    # odd tail: plain bf16 square + own-group identity matmul
                    sq84 = zp.tile([COMP, S], BF16, tag="z")
                    nc.scalar.activation(sq84[:], pr[:], AF.Square, scale=QSC)
                    pend.append((sq84, -1))
                else:
                    nc.scalar.activation(sq_cur[0][:, c % 2, :], pr[:],
                                         AF.Square, scale=QSC)
                    if c % 2 == 1:
                        pend.append((sq_cur[0], c // 2))
            for sqt, n in pend:
                if n >= 0:
                    acc_pair(sqt, n)
                else:
                    for ch in range(S // CH):
                        sl = slice(ch * CH, (ch + 1) * CH)
                        nc.tensor.matmul(accp[0:COMP, sl], w["idenb"][:], sqt[:, sl],
                                         start=True, stop=True,
                                         skip_group_check=True)
                    spill()

            # ---- finale: jn2 = ones^T (d3 * acc); out = tanh(kout*fout/(sqrt+eps)) ----
            am = zp.tile([COMP, S], BF16, tag="am")
            nc.vector.tensor_tensor(am[:], ACCs[:], d3[:], OP.mult)
            pj = psA.tile([1, S], F32, tag="a", name="pj")
            for ch in range(S // CH):
                sl = slice(ch * CH, (ch + 1) * CH)
                nc.tensor.matmul(pj[:, sl], w["onesb"][:], am[:, sl],
                                 start=True, stop=True)
            jn2 = ap.tile([1, S], F32, tag="jn2")
            nc.scalar.copy(jn2[:], pj[:])

            tc.strict_bb_all_engine_barrier()

            # batch the per-sample scale: transpose jn2/kout for all blocks
            # into one [128, 2*NB] tile, then one sqrt + vector recip pass.
            # jn2 holds (JSC*jn)^2, so den = JSC*jn and the reciprocal is
            # rescaled by JSC: scl = kout * JSC/(den + JSC*EPS).
            pjk = psA.tile([128, 2 * NB], F32, tag="a", name="pjk")
            for nb in range(NB):
                sl = slice(nb * 128, (nb + 1) * 128)
                nc.tensor.transpose(pjk[:, nb:nb + 1], jn2[:, sl], w["iden"][:1, :1])
                nc.tensor.transpose(pjk[:, NB + nb:NB + nb + 1], kout[:, sl],
                                    w["iden"][:1, :1])
            den = smp.tile([128, NB], F32, tag="den")
            nc.scalar.activation(den[:], pjk[:, 0:NB], AF.Sqrt)
            rec = smp.tile([128, NB], F32, tag="rec")
            nc.vector.tensor_scalar_add(rec[:], den[:], JSC * EPS)
            nc.vector.reciprocal(rec[:], rec[:])
            scl = smp.tile([128, NB], F32, tag="scl")
            nc.vector.tensor_scalar(out=scl[:], in0=rec[:], scalar1=JSC,
                                    scalar2=None, op0=OP.mult)
            nc.vector.tensor_tensor(scl[:], scl[:], pjk[:, NB:2 * NB], OP.mult)
            for nb in range(NB):
                sl = slice(nb * 128, (nb + 1) * 128)
                pt = psA.tile([128, COMP], F32, tag="a", name="ptf")
                nc.tensor.transpose(pt[:], fout[:, sl], w["iden"][:COMP, :COMP])
                ot = outp.tile([128, COMP], F32, tag="ot")
                nc.scalar.activation(ot[:], pt[:], AF.Tanh, scale=scl[:, nb:nb + 1])
                nc.sync.dma_start(out_d[sl, :], ot[:])

    return nc


_NC = None


def _get_nc():
    global _NC
    if _NC is None:
        _NC = build_nc()
        _NC.finalize()
    return _NC


def make_in_maps(inputs):
    w = host_prep(inputs)
    obs = np.ascontiguousarray(np.asarray(inputs["obs"], np.float32))
    act = np.ascontiguousarray(np.asarray(inputs["action"], np.float32))
    in_maps = []
    for i in range(NCORES):
        m = dict(w)
        m["obs"] = np.ascontiguousarray(obs[i * S:(i + 1) * S])
        m["action"] = np.ascontiguousarray(act[i * S:(i + 1) * S])
        m["tick"] = np.zeros((1, 1), np.float32)
        in_maps.append(m)
    return in_maps


def kernel(**inputs):
    from concourse.bass_utils import run_bass_kernel_spmd

    nc = _get_nc()
    in_maps = make_in_maps(inputs)
    res = run_bass_kernel_spmd(nc, in_maps, core_ids=list(range(NCORES)))
    return np.concatenate([r["out"] for r in res.results], axis=0)

